# revision 30
# baseline (speedup 1.0000x reference)
"""Trainium2 Bass kernel for nn_Attention_21792664060632.

GQA attention (32 q heads, 8 kv heads, d=64, s=2048, hidden=2048, causal,
interleaved RoPE) sharded tensor-parallel over 8 NeuronCores: core c owns q
heads {c, c+8, c+16, c+24} (all map to kv head c) plus kv head c.  Each core
computes a partial output projection and the host sums the 8 partials.

v2 design (all matmuls bf16, fp32 psum accumulate):
  - QKV^T formulation: Wcat chunks stationary, xT moving -> Q/K/V arrive
    TRANSPOSED ([dim, seq]) straight from the PE, no transposes needed for
    Q/K.  V is re-naturalized with 4 tiny bf16 PE transposes per 512-group.
  - Split-pair RoPE: host permutes Wq/Wk columns within each head to
    [evens | odds] (S = q.k is invariant under a shared permutation), so
    RoPE is 6 contiguous partition-block DVE ops per 128x512 chunk using
    host-built cosT/sinT [128, S] tables.
  - Causal staircase attention as before: S^T per 128-k-block with 2 heads
    on PE row-tiles (0,0)/(64,0), exp on ScalarE (merged [128,1024] where
    legal), AV with ones-row denominator trick, reciprocal straight off
    psum, gpsimd partition-broadcast, normalize to bf16.
  - Out projection bf16, psum evacuated by DVE to bf16, DMA'd out as bf16;
    host upcasts and sums partials in fp32.
"""

import sys

sys.path.insert(0, "/opt/trn_rl_repo")

import numpy as np

HEADS, KV_HEADS, HEAD_DIM = 32, 8, 64
S, HID = 2048, 2048
NCORES = 8
SC = S // 128  # 16 s-chunks
KC = HID // 128  # 16 hidden-chunks
NG = S // 512  # 4 seq groups / q-tiles

_CACHE = {}


def _build_nc(debug=False):
    import concourse.bacc as bacc
    import concourse.mybir as mybir
    import concourse.tile as tile
    from concourse.masks import make_identity

    F32 = mybir.dt.float32
    BF16 = mybir.dt.bfloat16
    EXP = mybir.ActivationFunctionType.Exp
    MULT = mybir.AluOpType.mult
    SUB = mybir.AluOpType.subtract
    ADD = mybir.AluOpType.add

    nc = bacc.Bacc("TRN2", target_bir_lowering=False, debug=False)

    XT = nc.dram_tensor("xt", [HID, S], BF16, kind="ExternalInput")
    WCAT = nc.dram_tensor("wcat", [HID, 384], BF16, kind="ExternalInput")
    WO = nc.dram_tensor("wo", [256, HID], BF16, kind="ExternalInput")
    COST = nc.dram_tensor("cost", [128, S], F32, kind="ExternalInput")
    SINT = nc.dram_tensor("sint", [128, S], F32, kind="ExternalInput")
    OUT = nc.dram_tensor("out", [S, HID], BF16, kind="ExternalOutput")
    if debug:
        DQT0 = nc.dram_tensor("dqt0", [128, S], BF16, kind="ExternalOutput")
        DQT1 = nc.dram_tensor("dqt1", [128, S], BF16, kind="ExternalOutput")
        DKT = nc.dram_tensor("dkt", [128, S], BF16, kind="ExternalOutput")
        DV = nc.dram_tensor("dv", [128, SC * 65], BF16, kind="ExternalOutput")
        DAVT0 = nc.dram_tensor("davt0", [128, S], BF16, kind="ExternalOutput")
        DAVT1 = nc.dram_tensor("davt1", [128, S], BF16, kind="ExternalOutput")

    with tile.TileContext(nc) as tc:
        with (
            tc.tile_pool(name="const", bufs=1) as const,
            tc.tile_pool(name="weights", bufs=1) as wpool,
            tc.tile_pool(name="persist", bufs=1) as persist,
        ):
            identf = const.tile([128, 128], F32)
            make_identity(nc, identf[:])
            ident = const.tile([128, 128], BF16)
            nc.vector.tensor_copy(ident[:], identf[:])
            mask01f = const.tile([128, 128], F32)
            nc.gpsimd.memset(mask01f[:], 1.0)
            # keep only q >= k: free index (q) >= partition index (k)
            nc.gpsimd.affine_select(
                out=mask01f[:], in_=mask01f[:],
                compare_op=mybir.AluOpType.is_ge,
                fill=0.0, base=0,
                pattern=[[1, 128]], channel_multiplier=-1,
            )
            mask01 = const.tile([128, 128], BF16)
            nc.vector.tensor_copy(mask01[:], mask01f[:])
            cosT = const.tile([128, S], F32)
            sinT = const.tile([128, S], F32)
            # pair-swap permutation matrix (split layout: swap 32-halves
            # within each 64-block): Pmat[32b+i, 32(b^1)+i] = 1
            pmat = const.tile([128, 128], BF16)
            nc.gpsimd.memset(pmat[:], 0.0)
            for b in range(4):
                r = (b ^ 1) * 32
                nc.vector.tensor_copy(
                    pmat[r:r + 32, b * 32:(b + 1) * 32], identf[0:32, 0:32]
                )

            # weight DMA in m-chunk priority order so the first QKV matmuls
            # can start as soon as m0's 16 kc-blocks land
            wcat_sb = wpool.tile([128, KC, 384], BF16)
            wo_sb = wpool.tile([128, 2, HID], BF16)
            wview = WCAT[:].rearrange("(c p) f -> p c f", p=128)
            for m in range(3):
                for kq in range(4):
                    nc.gpsimd.dma_start(
                        wcat_sb[:, kq * 4:(kq + 1) * 4, m * 128:(m + 1) * 128],
                        wview[:, kq * 4:(kq + 1) * 4, m * 128:(m + 1) * 128],
                    )
                if m == 0:
                    nc.gpsimd.dma_start(cosT[:], COST[:])
                    nc.gpsimd.dma_start(sinT[:], SINT[:])
            for c in range(2):
                nc.gpsimd.dma_start(wo_sb[:, c, :], WO[c * 128:(c + 1) * 128, :])

            # persistent transposed activations (bf16)
            qt0 = persist.tile([128, S], BF16)  # heads pair 0 on part 0:64/64:128
            qt1 = persist.tile([128, S], BF16)  # heads pair 1
            kt = persist.tile([128, S], BF16)  # kv head replicated in both halves
            v_sb = persist.tile([128, SC, 65], BF16)  # V natural + ones column
            nc.vector.memset(v_sb[:, :, 64:65], 1.0)
            avt0 = persist.tile([128, S], BF16)  # normalized attn out, pair 0
            avt1 = persist.tile([128, S], BF16)
            qts = [qt0, qt1]
            avts = [avt0, avt1]

            with (
                tc.tile_pool(name="xin", bufs=2) as xin,
                tc.tile_pool(name="ropet", bufs=3) as ropet,
                tc.tile_pool(name="pd", bufs=4) as pd,
                tc.tile_pool(name="nrm", bufs=2) as nrm,
                tc.tile_pool(name="ob", bufs=3) as obp,
                tc.tile_pool(name="proj", bufs=1, space="PSUM") as proj,
                tc.tile_pool(name="stp", bufs=2, space="PSUM") as stp,
                tc.tile_pool(name="psav", bufs=1, space="PSUM") as psav,
                tc.tile_pool(name="pso", bufs=1, space="PSUM") as pso,
            ):
                xt_grps = {}

                def prefetch_x(g):
                    xg = xin.tile([128, KC, 512], BF16, tag="xtg", name=f"xtg_{g}")
                    view = XT[:, g * 512:(g + 1) * 512].rearrange(
                        "(c p) s -> p c s", p=128
                    )
                    for q in range(4):
                        nc.sync.dma_start(
                            xg[:, q * 4:(q + 1) * 4, :], view[:, q * 4:(q + 1) * 4, :]
                        )
                    xt_grps[g] = xg

                def projrope_units(g):
                    """Per chunk m: 4 matmul units into a 1-bank psum tile,
                    then ropeA (psum readers: cast + cos-mul) and ropeB
                    (PE pair-swap + sin-mul + add).  Returns unit closures
                    in dependency order."""
                    xg = xt_grps[g]
                    sl = slice(g * 512, (g + 1) * 512)
                    units = []
                    for m in range(3):
                        pjc = proj.tile(
                            [128, 512], F32, tag="proj", name=f"proj_{g}_{m}"
                        )
                        for kh in range(4):
                            def u(m=m, kh=kh, pjc=pjc):
                                for kc in range(kh * 4, kh * 4 + 4):
                                    nc.tensor.matmul(
                                        pjc[:],
                                        wcat_sb[:, kc, m * 128:(m + 1) * 128],
                                        xg[:, kc, :],
                                        start=(kc == 0), stop=(kc == KC - 1),
                                    )
                            units.append(u)
                        # tan-rope: t1 = pj*cos; swap(t1) = swap(pj)*cos since
                        # the cos table is swap-invariant; then
                        # qrot = t1 + swap(t1)*tan  with tan = (+-)sin/cos.
                        t1 = ropet.tile([128, 512], BF16, tag="t1")
                        if m < 2:
                            def ropeA(pjc=pjc, t1=t1):
                                nc.vector.tensor_tensor(
                                    t1[:], pjc[:], cosT[:, sl], MULT)

                            def ropeB(m=m, t1=t1):
                                qsw = pso.tile([128, 512], F32, tag="pso")
                                nc.tensor.matmul(
                                    qsw[:], pmat[:], t1[:],
                                    start=True, stop=True)
                                t2 = ropet.tile([128, 512], BF16, tag="t2")
                                nc.vector.tensor_tensor(
                                    t2[:], qsw[:], sinT[:, sl], MULT)
                                nc.vector.tensor_tensor(
                                    qts[m][:, sl], t1[:], t2[:], ADD)
                        else:
                            vt = ropet.tile([64, 512], F32, tag="vt")

                            def ropeA(pjc=pjc, t1=t1, vt=vt):
                                nc.vector.tensor_tensor(
                                    t1[0:64, :], pjc[0:64, :], cosT[0:64, sl],
                                    MULT)
                                nc.vector.tensor_copy(vt[:], pjc[64:128, :])

                            def ropeB(t1=t1, vt=vt):
                                qsw = pso.tile([128, 512], F32, tag="pso")
                                nc.tensor.matmul(
                                    qsw[0:64, :], pmat[0:64, 0:64], t1[0:64, :],
                                    start=True, stop=True)
                                t2 = ropet.tile([128, 512], BF16, tag="t2")
                                nc.vector.tensor_tensor(
                                    t2[0:64, :], qsw[0:64, :], sinT[0:64, sl],
                                    MULT)
                                nc.vector.tensor_tensor(
                                    kt[0:64, sl], t1[0:64, :], t2[0:64, :], ADD)
                                nc.vector.tensor_tensor(
                                    kt[64:128, sl], t1[0:64, :], t2[0:64, :], ADD)
                                vps = pso.tile([128, 512], F32, tag="pso")
                                for i in range(4):
                                    nc.tensor.transpose(
                                        vps[:, i * 64:(i + 1) * 64],
                                        vt[:, i * 128:(i + 1) * 128],
                                        identf[0:64, 0:64],
                                    )
                                nc.vector.tensor_copy(
                                    v_sb[:, g * 4:(g + 1) * 4, 0:64],
                                    vps[:, 0:256].rearrange(
                                        "p (a b) -> p a b", a=4),
                                )
                        units.append(ropeA)
                        units.append(ropeB)
                    return units

                def attention_stage(g, fillers=()):
                    fillers = list(fillers)
                    n_units = 2 * (4 * g + 4)
                    nfl = len(fillers)
                    unit_idx = 0
                    popped = 0
                    q0 = g * 512
                    kimax = 4 * g + 3
                    for pr in range(2):
                        qt = qts[pr]
                        avp = psav.tile([65, 1024], F32, tag="avp")
                        avs = (avp[:, 0:512], avp[:, 512:1024])
                        for ki in range(kimax + 1):
                            d = ki - 4 * g
                            qoff = 0 if d < 0 else d * 128
                            st = stp.tile([128, 1024], F32, tag="st")
                            for h in range(2):
                                hp = h * 64
                                nc.tensor.matmul(
                                    st[:, h * 512 + qoff:h * 512 + 512],
                                    kt[hp:hp + 64, ki * 128:(ki + 1) * 128],
                                    qt[hp:hp + 64, q0 + qoff:q0 + 512],
                                    start=True, stop=True,
                                )
                            unit_idx += 1
                            want = nfl * unit_idx // n_units
                            while popped < want and fillers:
                                fillers.pop(0)()
                                popped += 1
                            p = pd.tile([128, 1024], BF16, tag="p")
                            if d <= 0:
                                nc.scalar.activation(p[:], st[:], EXP, scale=0.125)
                            else:
                                # one ACT instr over both heads' staircase
                                # slices via a 3D access pattern
                                stv = st[:].rearrange("x (h q) -> x h q", h=2)
                                pv = p[:].rearrange("x (h q) -> x h q", h=2)
                                nc.scalar.activation(
                                    pv[:, :, qoff:512],
                                    stv[:, :, qoff:512],
                                    EXP, scale=0.125,
                                )
                            if d >= 0:
                                pv = p[:].rearrange("x (h q) -> x h q", h=2)
                                nc.vector.tensor_tensor(
                                    pv[:, :, qoff:qoff + 128],
                                    pv[:, :, qoff:qoff + 128],
                                    mask01[:, None, :].to_broadcast([128, 2, 128]),
                                    MULT,
                                )
                            for h in range(2):
                                o = h * 512 + qoff
                                nc.tensor.matmul(
                                    avs[h][:, qoff:512],
                                    v_sb[:, ki, :],
                                    p[:, o:(h + 1) * 512],
                                    start=(ki == 0), stop=(ki == kimax),
                                )
                        # normalize: row 64 of av psum is the denominator
                        bc = nrm.tile([64, 1024], F32, tag="bc")
                        den = nrm.tile([1, 1024], F32, tag="den")
                        nc.vector.tensor_copy(den[:], avp[64:65, :])
                        rec = nrm.tile([1, 1024], F32, tag="rec")
                        nc.vector.reciprocal_approx_fast(rec[:], den[:])
                        nc.gpsimd.partition_broadcast(bc[:], rec[0:1, :])
                        for h in range(2):
                            hp = h * 64
                            nc.vector.tensor_tensor(
                                avts[pr][hp:hp + 64, g * 512:(g + 1) * 512],
                                avs[h][0:64, :], bc[:, h * 512:(h + 1) * 512],
                                MULT,
                            )
                    for f in fillers:
                        f()

                def out_units(g, tail=False):
                    """Closures: one per si -> 8 matmuls + evac + DMA.
                    Tail variant double-buffers via the freed stp tiles."""
                    units = []
                    for si in range(4 * g, 4 * g + 4):
                        def u(si=si):
                            osb = obp.tile(
                                [128, HID], BF16, tag="ob", name=f"ob_{si}"
                            )
                            if tail:
                                for half in range(2):
                                    ops = stp.tile([128, 1024], F32, tag="st")
                                    for nj2 in range(2):
                                        nj = half * 2 + nj2
                                        for prx in range(2):
                                            nc.tensor.matmul(
                                                ops[:, nj2 * 512:(nj2 + 1) * 512],
                                                avts[prx][:, si * 128:(si + 1) * 128],
                                                wo_sb[:, prx, nj * 512:(nj + 1) * 512],
                                                start=(prx == 0), stop=(prx == 1),
                                            )
                                    nc.vector.tensor_copy(
                                        osb[:, half * 1024:(half + 1) * 1024],
                                        ops[:],
                                    )
                                    for nj2 in range(2):
                                        nj = half * 2 + nj2
                                        eng = nc.sync if nj % 2 else nc.gpsimd
                                        eng.dma_start(
                                            OUT[si * 128:(si + 1) * 128,
                                                nj * 512:(nj + 1) * 512],
                                            osb[:, nj * 512:(nj + 1) * 512],
                                        )
                            else:
                                for nj in range(4):
                                    ops = pso.tile([128, 512], F32, tag="pso")
                                    for prx in range(2):
                                        nc.tensor.matmul(
                                            ops[:],
                                            avts[prx][:, si * 128:(si + 1) * 128],
                                            wo_sb[:, prx, nj * 512:(nj + 1) * 512],
                                            start=(prx == 0), stop=(prx == 1),
                                        )
                                    nc.vector.tensor_copy(
                                        osb[:, nj * 512:(nj + 1) * 512], ops[:]
                                    )
                                    eng = nc.sync if nj % 2 else nc.gpsimd
                                    eng.dma_start(
                                        OUT[si * 128:(si + 1) * 128,
                                            nj * 512:(nj + 1) * 512],
                                        osb[:, nj * 512:(nj + 1) * 512],
                                    )
                        units.append(u)
                    return units

                # schedule: head = qkv+rope(0); per group: attention(g)
                # interleaved with [qkv+rope(g+1), out(g-1)]; tail = out(3)
                prefetch_x(0)
                for u in projrope_units(0):
                    u()
                for g in range(NG):
                    fillers = []
                    if g < NG - 1:
                        prefetch_x(g + 1)
                        pr_units = projrope_units(g + 1)
                        ou = out_units(g - 1) if g > 0 else []
                        # spread: after each chunk's ropeA, slot an out unit
                        for m in range(3):
                            fillers += pr_units[m * 6:m * 6 + 5]
                            if ou:
                                fillers.append(ou.pop(0))
                            fillers.append(pr_units[m * 6 + 5])
                        fillers += ou
                    else:
                        fillers = out_units(g - 1)
                    attention_stage(g, fillers)
                for u in out_units(NG - 1, tail=True):
                    u()
                if debug:
                    nc.gpsimd.dma_start(DQT0[:], qt0[:])
                    nc.gpsimd.dma_start(DQT1[:], qt1[:])
                    nc.gpsimd.dma_start(DKT[:], kt[:])
                    nc.gpsimd.dma_start(
                        DV[:], v_sb[:].rearrange("p a b -> p (a b)")
                    )
                    nc.gpsimd.dma_start(DAVT0[:], avt0[:])
                    nc.gpsimd.dma_start(DAVT1[:], avt1[:])

    nc.compile()
    return nc


def _shard_inputs(x, cos, sin, Wq, Wk, Wv, Wo):
    """Build the 8 per-core input maps (tensor-parallel by head groups)."""
    import ml_dtypes

    bf16 = ml_dtypes.bfloat16
    xt = np.ascontiguousarray(x.T).astype(bf16)
    # split-pair permutation within each head: [evens | odds]
    perm = np.concatenate([np.arange(0, 64, 2), np.arange(1, 64, 2)])
    cosT = np.ascontiguousarray(np.tile(cos.T, (4, 1))).astype(np.float32)
    # sign-folded tan table for the PE pair-swap tan-rope:
    # rows [0:32] -sin/cos, [32:64] +sin/cos, repeating
    tanb = np.concatenate([-sin.T, sin.T], axis=0) / np.tile(cos.T, (2, 1))
    sinT = np.ascontiguousarray(np.tile(tanb, (2, 1))).astype(np.float32)
    in_maps = []
    for c in range(NCORES):
        heads = [c, c + 8, c + 16, c + 24]
        wq_cols = np.concatenate(
            [Wq[:, h * 64:(h + 1) * 64][:, perm] for h in heads], axis=1
        )
        wcat = np.concatenate(
            [
                wq_cols,
                Wk[:, c * 64:(c + 1) * 64][:, perm],
                Wv[:, c * 64:(c + 1) * 64],
            ],
            axis=1,
        )
        wo_rows = np.concatenate(
            [Wo[h * 64:(h + 1) * 64, :] for h in heads], axis=0
        )
        in_maps.append(
            {
                "xt": xt,
                "wcat": np.ascontiguousarray(wcat).astype(bf16),
                "wo": np.ascontiguousarray(wo_rows).astype(bf16),
                "cost": cosT,
                "sint": sinT,
            }
        )
    return in_maps


def run(inputs, trace=False, debug=False):
    """Run on all 8 cores; returns (full_output [1,S,HID], BassKernelResults)."""
    from concourse.bass_utils import run_bass_kernel_spmd

    x = np.asarray(inputs["x"], dtype=np.float32)[0]
    cos = np.asarray(inputs["cos"], dtype=np.float32)
    sin = np.asarray(inputs["sin"], dtype=np.float32)
    Wq = np.asarray(inputs["Wq"], dtype=np.float32)
    Wk = np.asarray(inputs["Wk"], dtype=np.float32)
    Wv = np.asarray(inputs["Wv"], dtype=np.float32)
    Wo = np.asarray(inputs["Wo"], dtype=np.float32)

    key = ("nc", debug)
    if key not in _CACHE:
        _CACHE[key] = _build_nc(debug=debug)
    nc = _CACHE[key]

    in_maps = _shard_inputs(x, cos, sin, Wq, Wk, Wv, Wo)
    res = run_bass_kernel_spmd(
        nc, in_maps, core_ids=list(range(NCORES)), trace=trace
    )
    out = np.zeros((S, HID), dtype=np.float32)
    for r in res.results:
        out += r["out"].astype(np.float32)
    return out.reshape(1, S, HID), res


def kernel(**inputs) -> np.ndarray:
    out, _ = run(inputs, trace=False)
    return out


# revision 31
# speedup vs baseline: 1.2177x; 1.2177x over previous
"""Trainium2 Bass kernel for nn_Attention_21792664060632.

GQA attention (32 q heads, 8 kv heads, d=64, s=2048, hidden=2048, causal,
interleaved RoPE) sharded tensor-parallel over 8 NeuronCores: core c owns q
heads {c, c+8, c+16, c+24} (all map to kv head c) plus kv head c.  Each core
computes a partial output projection and the host sums the 8 partials.

v2 design (all matmuls bf16, fp32 psum accumulate):
  - QKV^T formulation: Wcat chunks stationary, xT moving -> Q/K/V arrive
    TRANSPOSED ([dim, seq]) straight from the PE, no transposes needed for
    Q/K.  V is re-naturalized with 4 tiny bf16 PE transposes per 512-group.
  - Split-pair RoPE: host permutes Wq/Wk columns within each head to
    [evens | odds] (S = q.k is invariant under a shared permutation), so
    RoPE is 6 contiguous partition-block DVE ops per 128x512 chunk using
    host-built cosT/sinT [128, S] tables.
  - Causal staircase attention as before: S^T per 128-k-block with 2 heads
    on PE row-tiles (0,0)/(64,0), exp on ScalarE (merged [128,1024] where
    legal), AV with ones-row denominator trick, reciprocal straight off
    psum, gpsimd partition-broadcast, normalize to bf16.
  - Out projection bf16, psum evacuated by DVE to bf16, DMA'd out as bf16;
    host upcasts and sums partials in fp32.
"""

import sys

sys.path.insert(0, "/opt/trn_rl_repo")

import numpy as np

HEADS, KV_HEADS, HEAD_DIM = 32, 8, 64
S, HID = 2048, 2048
NCORES = 8
SC = S // 128  # 16 s-chunks
KC = HID // 128  # 16 hidden-chunks
NG = S // 512  # 4 seq groups / q-tiles

_CACHE = {}


def _build_nc(debug=False):
    import concourse.bacc as bacc
    import concourse.mybir as mybir
    import concourse.tile as tile
    from concourse.masks import make_identity

    F32 = mybir.dt.float32
    BF16 = mybir.dt.bfloat16
    EXP = mybir.ActivationFunctionType.Exp
    MULT = mybir.AluOpType.mult
    SUB = mybir.AluOpType.subtract
    ADD = mybir.AluOpType.add

    nc = bacc.Bacc("TRN2", target_bir_lowering=False, debug=False)

    XT = nc.dram_tensor("xt", [HID, S], BF16, kind="ExternalInput")
    WCAT = nc.dram_tensor("wcat", [HID, 384], BF16, kind="ExternalInput")
    WO = nc.dram_tensor("wo", [256, HID], BF16, kind="ExternalInput")
    COST = nc.dram_tensor("cost", [128, S], F32, kind="ExternalInput")
    SINT = nc.dram_tensor("sint", [128, S], F32, kind="ExternalInput")
    OUT = nc.dram_tensor("out", [S, HID], BF16, kind="ExternalOutput")
    if debug:
        DQT0 = nc.dram_tensor("dqt0", [128, S], BF16, kind="ExternalOutput")
        DQT1 = nc.dram_tensor("dqt1", [128, S], BF16, kind="ExternalOutput")
        DKT = nc.dram_tensor("dkt", [128, S], BF16, kind="ExternalOutput")
        DV = nc.dram_tensor("dv", [128, SC * 65], BF16, kind="ExternalOutput")
        DAVT0 = nc.dram_tensor("davt0", [128, S], BF16, kind="ExternalOutput")
        DAVT1 = nc.dram_tensor("davt1", [128, S], BF16, kind="ExternalOutput")

    with tile.TileContext(nc) as tc:
        with (
            tc.tile_pool(name="const", bufs=1) as const,
            tc.tile_pool(name="weights", bufs=1) as wpool,
            tc.tile_pool(name="persist", bufs=1) as persist,
        ):
            identf = const.tile([128, 128], F32)
            make_identity(nc, identf[:])
            ident = const.tile([128, 128], BF16)
            nc.vector.tensor_copy(ident[:], identf[:])
            mask01f = const.tile([128, 128], F32)
            nc.gpsimd.memset(mask01f[:], 1.0)
            # keep only q >= k: free index (q) >= partition index (k)
            nc.gpsimd.affine_select(
                out=mask01f[:], in_=mask01f[:],
                compare_op=mybir.AluOpType.is_ge,
                fill=0.0, base=0,
                pattern=[[1, 128]], channel_multiplier=-1,
            )
            mask01 = const.tile([128, 128], BF16)
            nc.vector.tensor_copy(mask01[:], mask01f[:])
            cosT = const.tile([128, S], F32)
            sinT = const.tile([128, S], F32)
            # pair-swap permutation matrix (split layout: swap 32-halves
            # within each 64-block): Pmat[32b+i, 32(b^1)+i] = 1
            pmat = const.tile([128, 128], BF16)
            nc.gpsimd.memset(pmat[:], 0.0)
            for b in range(4):
                r = (b ^ 1) * 32
                nc.vector.tensor_copy(
                    pmat[r:r + 32, b * 32:(b + 1) * 32], identf[0:32, 0:32]
                )

            # weight DMA in m-chunk priority order so the first QKV matmuls
            # can start as soon as m0's 16 kc-blocks land
            wcat_sb = wpool.tile([128, KC, 384], BF16)
            wo_sb = wpool.tile([128, 2, HID], BF16)
            wview = WCAT[:].rearrange("(c p) f -> p c f", p=128)
            for m in range(3):
                for kq in range(4):
                    nc.gpsimd.dma_start(
                        wcat_sb[:, kq * 4:(kq + 1) * 4, m * 128:(m + 1) * 128],
                        wview[:, kq * 4:(kq + 1) * 4, m * 128:(m + 1) * 128],
                    )
                if m == 0:
                    nc.gpsimd.dma_start(cosT[:], COST[:])
                    nc.gpsimd.dma_start(sinT[:], SINT[:])
            for c in range(2):
                nc.gpsimd.dma_start(wo_sb[:, c, :], WO[c * 128:(c + 1) * 128, :])

            # persistent transposed activations (bf16)
            qt0 = persist.tile([128, S], BF16)  # heads pair 0 on part 0:64/64:128
            qt1 = persist.tile([128, S], BF16)  # heads pair 1
            kt = persist.tile([128, S], BF16)  # kv head replicated in both halves
            v_sb = persist.tile([128, SC, 65], BF16)  # V natural + ones column
            nc.vector.memset(v_sb[:, :, 64:65], 1.0)
            avt0 = persist.tile([128, S], BF16)  # normalized attn out, pair 0
            avt1 = persist.tile([128, S], BF16)
            qts = [qt0, qt1]
            avts = [avt0, avt1]

            with (
                tc.tile_pool(name="xin", bufs=2) as xin,
                tc.tile_pool(name="ropet", bufs=3) as ropet,
                tc.tile_pool(name="pd", bufs=4) as pd,
                tc.tile_pool(name="nrm", bufs=2) as nrm,
                tc.tile_pool(name="ob", bufs=3) as obp,
                tc.tile_pool(name="proj", bufs=1, space="PSUM") as proj,
                tc.tile_pool(name="stp", bufs=2, space="PSUM") as stp,
                tc.tile_pool(name="psav", bufs=1, space="PSUM") as psav,
                tc.tile_pool(name="pso", bufs=1, space="PSUM") as pso,
            ):
                xt_grps = {}

                def prefetch_x(g):
                    xg = xin.tile([128, KC, 512], BF16, tag="xtg", name=f"xtg_{g}")
                    view = XT[:, g * 512:(g + 1) * 512].rearrange(
                        "(c p) s -> p c s", p=128
                    )
                    for q in range(4):
                        nc.sync.dma_start(
                            xg[:, q * 4:(q + 1) * 4, :], view[:, q * 4:(q + 1) * 4, :]
                        )
                    xt_grps[g] = xg

                def projrope_units(g):
                    """Per chunk m: 4 matmul units into a 1-bank psum tile,
                    then ropeA (psum readers: cast + cos-mul) and ropeB
                    (PE pair-swap + sin-mul + add).  Returns unit closures
                    in dependency order."""
                    xg = xt_grps[g]
                    sl = slice(g * 512, (g + 1) * 512)
                    units = []
                    for m in range(3):
                        pjc = proj.tile(
                            [128, 512], F32, tag="proj", name=f"proj_{g}_{m}"
                        )
                        for kh in range(4):
                            def u(m=m, kh=kh, pjc=pjc):
                                for kc in range(kh * 4, kh * 4 + 4):
                                    nc.tensor.matmul(
                                        pjc[:],
                                        wcat_sb[:, kc, m * 128:(m + 1) * 128],
                                        xg[:, kc, :],
                                        start=(kc == 0), stop=(kc == KC - 1),
                                    )
                            units.append(u)
                        # tan-rope: t1 = pj*cos; swap(t1) = swap(pj)*cos since
                        # the cos table is swap-invariant; then
                        # qrot = t1 + swap(t1)*tan  with tan = (+-)sin/cos.
                        t1 = ropet.tile([128, 512], BF16, tag="t1")
                        if m < 2:
                            def ropeA(pjc=pjc, t1=t1):
                                nc.vector.tensor_tensor(
                                    t1[:], pjc[:], cosT[:, sl], MULT)

                            def ropeB(m=m, t1=t1):
                                qsw = pso.tile([128, 512], F32, tag="pso")
                                nc.tensor.matmul(
                                    qsw[:], pmat[:], t1[:],
                                    start=True, stop=True)
                                t2 = ropet.tile([128, 512], BF16, tag="t2")
                                nc.vector.tensor_tensor(
                                    t2[:], qsw[:], sinT[:, sl], MULT)
                                nc.vector.tensor_tensor(
                                    qts[m][:, sl], t1[:], t2[:], ADD)
                        else:
                            vt = ropet.tile([64, 512], F32, tag="vt")

                            def ropeA(pjc=pjc, t1=t1, vt=vt):
                                nc.vector.tensor_tensor(
                                    t1[0:64, :], pjc[0:64, :], cosT[0:64, sl],
                                    MULT)
                                nc.vector.tensor_copy(vt[:], pjc[64:128, :])

                            def ropeB(t1=t1, vt=vt):
                                qsw = pso.tile([128, 512], F32, tag="pso")
                                nc.tensor.matmul(
                                    qsw[0:64, :], pmat[0:64, 0:64], t1[0:64, :],
                                    start=True, stop=True)
                                t2 = ropet.tile([128, 512], BF16, tag="t2")
                                nc.vector.tensor_tensor(
                                    t2[0:64, :], qsw[0:64, :], sinT[0:64, sl],
                                    MULT)
                                nc.vector.tensor_tensor(
                                    kt[0:64, sl], t1[0:64, :], t2[0:64, :], ADD)
                                nc.vector.tensor_tensor(
                                    kt[64:128, sl], t1[0:64, :], t2[0:64, :], ADD)
                                vps = pso.tile([128, 512], F32, tag="pso")
                                for i in range(4):
                                    nc.tensor.transpose(
                                        vps[:, i * 64:(i + 1) * 64],
                                        vt[:, i * 128:(i + 1) * 128],
                                        identf[0:64, 0:64],
                                    )
                                nc.vector.tensor_copy(
                                    v_sb[:, g * 4:(g + 1) * 4, 0:64],
                                    vps[:, 0:256].rearrange(
                                        "p (a b) -> p a b", a=4),
                                )
                        units.append(ropeA)
                        units.append(ropeB)
                    return units

                def attention_stage(g, fillers=()):
                    fillers = list(fillers)
                    n_units = 2 * (4 * g + 4)
                    nfl = len(fillers)
                    unit_idx = 0
                    popped = 0
                    q0 = g * 512
                    kimax = 4 * g + 3
                    for pr in range(2):
                        qt = qts[pr]
                        avp = psav.tile([65, 1024], F32, tag="avp")
                        avs = (avp[:, 0:512], avp[:, 512:1024])
                        for ki in range(kimax + 1):
                            d = ki - 4 * g
                            qoff = 0 if d < 0 else d * 128
                            st = stp.tile([128, 1024], F32, tag="st")
                            for h in range(2):
                                hp = h * 64
                                nc.tensor.matmul(
                                    st[:, h * 512 + qoff:h * 512 + 512],
                                    kt[hp:hp + 64, ki * 128:(ki + 1) * 128],
                                    qt[hp:hp + 64, q0 + qoff:q0 + 512],
                                    start=True, stop=True,
                                )
                            unit_idx += 1
                            want = nfl * unit_idx // n_units
                            while popped < want and fillers:
                                fillers.pop(0)()
                                popped += 1
                            p = pd.tile([128, 1024], BF16, tag="p")
                            if d <= 0:
                                nc.scalar.activation(p[:], st[:], EXP, scale=0.125)
                            else:
                                # one ACT instr over both heads' staircase
                                # slices via a 3D access pattern
                                stv = st[:].rearrange("x (h q) -> x h q", h=2)
                                pv = p[:].rearrange("x (h q) -> x h q", h=2)
                                nc.scalar.activation(
                                    pv[:, :, qoff:512],
                                    stv[:, :, qoff:512],
                                    EXP, scale=0.125,
                                )
                            if d >= 0:
                                pv = p[:].rearrange("x (h q) -> x h q", h=2)
                                nc.vector.tensor_tensor(
                                    pv[:, :, qoff:qoff + 128],
                                    pv[:, :, qoff:qoff + 128],
                                    mask01[:, None, :].to_broadcast([128, 2, 128]),
                                    MULT,
                                )
                            for h in range(2):
                                o = h * 512 + qoff
                                nc.tensor.matmul(
                                    avs[h][:, qoff:512],
                                    v_sb[:, ki, :],
                                    p[:, o:(h + 1) * 512],
                                    start=(ki == 0), stop=(ki == kimax),
                                )
                        # normalize: row 64 of av psum is the denominator
                        bc = nrm.tile([64, 1024], F32, tag="bc")
                        den = nrm.tile([1, 1024], F32, tag="den")
                        nc.vector.tensor_copy(den[:], avp[64:65, :])
                        rec = nrm.tile([1, 1024], F32, tag="rec")
                        nc.vector.reciprocal_approx_fast(rec[:], den[:])
                        nc.gpsimd.partition_broadcast(bc[:], rec[0:1, :])
                        for h in range(2):
                            hp = h * 64
                            nc.vector.tensor_tensor(
                                avts[pr][hp:hp + 64, g * 512:(g + 1) * 512],
                                avs[h][0:64, :], bc[:, h * 512:(h + 1) * 512],
                                MULT,
                            )
                    for f in fillers:
                        f()

                def out_units(g, tail=False):
                    """Closures: one per si -> 8 matmuls + evac + DMA.
                    Tail variant double-buffers via the freed stp tiles."""
                    units = []
                    for si in range(4 * g, 4 * g + 4):
                        def u(si=si):
                            osb = obp.tile(
                                [128, HID], BF16, tag="ob", name=f"ob_{si}"
                            )
                            if tail:
                                for half in range(2):
                                    ops = stp.tile([128, 1024], F32, tag="st")
                                    for nj2 in range(2):
                                        nj = half * 2 + nj2
                                        for prx in range(2):
                                            nc.tensor.matmul(
                                                ops[:, nj2 * 512:(nj2 + 1) * 512],
                                                avts[prx][:, si * 128:(si + 1) * 128],
                                                wo_sb[:, prx, nj * 512:(nj + 1) * 512],
                                                start=(prx == 0), stop=(prx == 1),
                                            )
                                    nc.vector.tensor_copy(
                                        osb[:, half * 1024:(half + 1) * 1024],
                                        ops[:],
                                    )
                                    for nj2 in range(2):
                                        nj = half * 2 + nj2
                                        eng = nc.sync if nj % 2 else nc.gpsimd
                                        eng.dma_start(
                                            OUT[si * 128:(si + 1) * 128,
                                                nj * 512:(nj + 1) * 512],
                                            osb[:, nj * 512:(nj + 1) * 512],
                                        )
                            else:
                                for nj in range(4):
                                    ops = pso.tile([128, 512], F32, tag="pso")
                                    for prx in range(2):
                                        nc.tensor.matmul(
                                            ops[:],
                                            avts[prx][:, si * 128:(si + 1) * 128],
                                            wo_sb[:, prx, nj * 512:(nj + 1) * 512],
                                            start=(prx == 0), stop=(prx == 1),
                                        )
                                    nc.vector.tensor_copy(
                                        osb[:, nj * 512:(nj + 1) * 512], ops[:]
                                    )
                                nc.gpsimd.dma_start(
                                    OUT[si * 128:(si + 1) * 128, :], osb[:]
                                )
                        units.append(u)
                    return units

                # schedule: head = qkv+rope(0); per group: attention(g)
                # interleaved with [qkv+rope(g+1), out(g-1)]; tail = out(3)
                prefetch_x(0)
                for u in projrope_units(0):
                    u()
                for g in range(NG):
                    fillers = []
                    if g < NG - 1:
                        prefetch_x(g + 1)
                        pr_units = projrope_units(g + 1)
                        ou = out_units(g - 1) if g > 0 else []
                        # spread: after each chunk's ropeA, slot an out unit
                        for m in range(3):
                            fillers += pr_units[m * 6:m * 6 + 5]
                            if ou:
                                fillers.append(ou.pop(0))
                            fillers.append(pr_units[m * 6 + 5])
                        fillers += ou
                    else:
                        fillers = out_units(g - 1)
                    attention_stage(g, fillers)
                for u in out_units(NG - 1, tail=True):
                    u()
                if debug:
                    nc.gpsimd.dma_start(DQT0[:], qt0[:])
                    nc.gpsimd.dma_start(DQT1[:], qt1[:])
                    nc.gpsimd.dma_start(DKT[:], kt[:])
                    nc.gpsimd.dma_start(
                        DV[:], v_sb[:].rearrange("p a b -> p (a b)")
                    )
                    nc.gpsimd.dma_start(DAVT0[:], avt0[:])
                    nc.gpsimd.dma_start(DAVT1[:], avt1[:])

    nc.compile()
    return nc


def _shard_inputs(x, cos, sin, Wq, Wk, Wv, Wo):
    """Build the 8 per-core input maps (tensor-parallel by head groups)."""
    import ml_dtypes

    bf16 = ml_dtypes.bfloat16
    xt = np.ascontiguousarray(x.T).astype(bf16)
    # split-pair permutation within each head: [evens | odds]
    perm = np.concatenate([np.arange(0, 64, 2), np.arange(1, 64, 2)])
    cosT = np.ascontiguousarray(np.tile(cos.T, (4, 1))).astype(np.float32)
    # sign-folded tan table for the PE pair-swap tan-rope:
    # rows [0:32] -sin/cos, [32:64] +sin/cos, repeating
    tanb = np.concatenate([-sin.T, sin.T], axis=0) / np.tile(cos.T, (2, 1))
    sinT = np.ascontiguousarray(np.tile(tanb, (2, 1))).astype(np.float32)
    in_maps = []
    for c in range(NCORES):
        heads = [c, c + 8, c + 16, c + 24]
        wq_cols = np.concatenate(
            [Wq[:, h * 64:(h + 1) * 64][:, perm] for h in heads], axis=1
        )
        wcat = np.concatenate(
            [
                wq_cols,
                Wk[:, c * 64:(c + 1) * 64][:, perm],
                Wv[:, c * 64:(c + 1) * 64],
            ],
            axis=1,
        )
        wo_rows = np.concatenate(
            [Wo[h * 64:(h + 1) * 64, :] for h in heads], axis=0
        )
        in_maps.append(
            {
                "xt": xt,
                "wcat": np.ascontiguousarray(wcat).astype(bf16),
                "wo": np.ascontiguousarray(wo_rows).astype(bf16),
                "cost": cosT,
                "sint": sinT,
            }
        )
    return in_maps


def run(inputs, trace=False, debug=False):
    """Run on all 8 cores; returns (full_output [1,S,HID], BassKernelResults)."""
    from concourse.bass_utils import run_bass_kernel_spmd

    x = np.asarray(inputs["x"], dtype=np.float32)[0]
    cos = np.asarray(inputs["cos"], dtype=np.float32)
    sin = np.asarray(inputs["sin"], dtype=np.float32)
    Wq = np.asarray(inputs["Wq"], dtype=np.float32)
    Wk = np.asarray(inputs["Wk"], dtype=np.float32)
    Wv = np.asarray(inputs["Wv"], dtype=np.float32)
    Wo = np.asarray(inputs["Wo"], dtype=np.float32)

    key = ("nc", debug)
    if key not in _CACHE:
        _CACHE[key] = _build_nc(debug=debug)
    nc = _CACHE[key]

    in_maps = _shard_inputs(x, cos, sin, Wq, Wk, Wv, Wo)
    res = run_bass_kernel_spmd(
        nc, in_maps, core_ids=list(range(NCORES)), trace=trace
    )
    out = np.zeros((S, HID), dtype=np.float32)
    for r in res.results:
        out += r["out"].astype(np.float32)
    return out.reshape(1, S, HID), res


def kernel(**inputs) -> np.ndarray:
    out, _ = run(inputs, trace=False)
    return out
